# revision 3
# baseline (speedup 1.0000x reference)
"""Trainium2 Bass kernel for a contextual loss (cosine-distance softmin loss).

Math (per batch b):
  mu_c      = mean_n Y[b,c,n]
  xc = X-mu, yc = Y-mu                      (centered, [C,N])
  t[i,j]    = <xc_i, yc_j/||yc_j||>         (bf16 matmul, K=C=64)
  s[i,j]    = rx_i * t[i,j]                 (rx = 1/||xc_i||)
  pm_i      = max_j t[i,j]
  a_i       = rx5_i / (1.001 - 5*rx5_i*pm_i)     (rx5 = 0.2*rx)
  S'_i      = sum_j exp(a_i*(t_ij - pm_i))
  loss_b    = -log(mean_i 1/S'_i)

Sharding: 8 cores = 4 batches x 2 row-halves. Each core gets its full-batch
Y [64,4096] and its half of X's columns [64,2048], returns S' as [128,16]
(partition p, chunk k  <->  row k*128+p). Host reduces to the [4] loss.

Column norms without pre-centering: ||y_j - mu||^2 = colsum(y^2) - 2*mu^T y_j
+ ||mu||^2 (the ||mu||^2 term is ~2e-4 relative, dropped).  Computed by PSUM
accumulation: matmul(ones^T @ y^2) then matmul((-2mu)^T @ y, start=False).

On-device pipeline per 128-row chunk:
  PE   : 8 bf16 matmuls (K=64, N=512) -> PSUM f32 [128,2048] x2
  DVE  : TENSOR_MASK_REDUCE fuses PSUM->SBUF copy with a running row-max
  GPSIMD + DVE: tiny per-row chain  den -> 1/den -> aa -> bb
  ACT  : one exp(aa*t + bb) over [128,4096] with accumulated row-sum
"""

import math

import numpy as np

import concourse.bacc as bacc
import concourse.mybir as mybir
from concourse.dve_ops import TENSOR_MASK_REDUCE
from concourse.bass_utils import run_bass_kernel_spmd
from concourse.mybir import ActivationFunctionType as AF, AluOpType as OP, AxisListType
from concourse.tile import TileContext

F32 = mybir.dt.float32
BF16 = mybir.dt.bfloat16

B, C, N = 4, 64, 4096          # batch, channels, spatial (64*64)
NX = N // 2                    # rows per core (half batch)
CH = NX // 128                 # 16 chunks of 128 rows
HALF = N // 2                  # column half processed per DVE op
H_BAND = 5.0
EPS_MIN = 1e-3
LN02 = math.log(0.2)           # fold the 1/H into rx via exp(... + ln(1/H))

_NC_CACHE = {}


def build_nc():
    nc = bacc.Bacc("TRN2", target_bir_lowering=False, debug=False, num_devices=8)
    x_d = nc.dram_tensor("Xh", [C, NX], F32, kind="ExternalInput")
    y_d = nc.dram_tensor("Yb", [C, N], F32, kind="ExternalInput")
    out_d = nc.dram_tensor("out", [128, CH], F32, kind="ExternalOutput")

    with TileContext(nc) as tc:
        with (
            tc.tile_pool(name="persist", bufs=1) as persist,
            tc.tile_pool(name="mm", bufs=2, space="PSUM") as mmpool,
            tc.tile_pool(name="rb", bufs=2) as rbpool,
            tc.tile_pool(name="es", bufs=2) as espool,
            tc.tile_pool(name="small", bufs=4) as small,
        ):
            # ---------------- load inputs ----------------
            y_sb = persist.tile([C, N], F32)
            NSL = 4
            for sl in range(NSL):
                c0 = sl * (N // NSL)
                nc.sync.dma_start(out=y_sb[:, c0:c0 + N // NSL],
                                  in_=y_d[:, c0:c0 + N // NSL])
            x_sb = persist.tile([C, NX], F32)
            nc.sync.dma_start(out=x_sb[:], in_=x_d[:])

            ones_f = persist.tile([C, 128], F32)
            nc.vector.memset(ones_f[:], 1.0)
            c3big = persist.tile([128, 1], F32)
            nc.gpsimd.memset(c3big[:], 1.0e9)
            ln02 = persist.tile([128, 1], F32)
            nc.gpsimd.memset(ln02[:], LN02)

            # ---------------- y mean (overlapped with DMA slices) -------------
            mus = small.tile([C, NSL], F32, tag="mus")
            for sl in range(NSL):
                c0 = sl * (N // NSL)
                nc.vector.reduce_sum(out=mus[:, sl:sl + 1],
                                     in_=y_sb[:, c0:c0 + N // NSL],
                                     axis=AxisListType.X)
            musum = small.tile([C, 1], F32, tag="musum")
            nc.vector.reduce_sum(out=musum[:], in_=mus[:], axis=AxisListType.X)
            mu = small.tile([C, 1], F32, tag="mu")
            nc.vector.tensor_scalar_mul(mu[:], musum[:], 1.0 / N)

            # -2*mu broadcast along free dim (for the colsum-correction mms)
            mu2bc = persist.tile([C, 128], F32)
            nc.vector.tensor_scalar(mu2bc[:], ones_f[:], mu[:], -2.0,
                                    OP.mult, OP.mult)

            # squares of raw y/x (ACT, no mu dependency)
            ysq = persist.tile([C, N], F32)
            for h in range(2):
                nc.scalar.activation(
                    ysq[:, h * HALF:(h + 1) * HALF],
                    y_sb[:, h * HALF:(h + 1) * HALF], AF.Square,
                )
            xsq = persist.tile([C, NX], F32)
            nc.scalar.activation(xsq[:], x_sb[:], AF.Square)

            # ---------------- ry broadcast -> yhat (bf16) ----------------
            # ny2_j = colsum(y^2) - 2 mu^T y_j via PSUM accumulation,
            # replicated down 128 partitions; ry = exp(-0.5*ln(ny2)).
            yhat = persist.tile([C, N], BF16)
            ry_bc = persist.tile([128, N], F32)
            for h in range(2):
                ps = mmpool.tile([128, HALF], F32, tag="mm")
                for j in range(4):
                    c0 = h * HALF + j * 512
                    nc.tensor.matmul(
                        ps[:, j * 512:(j + 1) * 512],
                        lhsT=ones_f[:],
                        rhs=ysq[:, c0:c0 + 512],
                        start=True, stop=False,
                    )
                    nc.tensor.matmul(
                        ps[:, j * 512:(j + 1) * 512],
                        lhsT=mu2bc[:],
                        rhs=y_sb[:, c0:c0 + 512],
                        start=False, stop=True,
                    )
                tln = espool.tile([128, HALF], F32, tag="es")
                nc.scalar.activation(tln[:], ps[:], AF.Ln)
                nc.scalar.activation(
                    ry_bc[:, h * HALF:(h + 1) * HALF], tln[:], AF.Exp, scale=-0.5
                )
                # yhat half: (y - mu) * ry, cast to bf16, in quarters so the
                # first main-loop matmuls can start early
                QW = HALF // 2
                for q in range(2):
                    c0 = h * HALF + q * QW
                    nc.vector.scalar_tensor_tensor(
                        yhat[:, c0:c0 + QW],
                        in0=y_sb[:, c0:c0 + QW],
                        scalar=mu[:],
                        in1=ry_bc[:C, c0:c0 + QW],
                        op0=OP.subtract,
                        op1=OP.mult,
                    )

            # ---------------- x side: xcen (bf16) + rx5 ----------------
            xcen = persist.tile([C, NX], BF16)
            nc.vector.tensor_scalar(xcen[:], x_sb[:], mu[:], None, OP.subtract)

            # nx2 = ||x_i - mu||^2 in [128 rows, chunk] layout, PSUM-accumulated
            nx2 = mmpool.tile([128, 2 * CH], F32, tag="mm")
            for k in range(CH):
                nc.tensor.matmul(
                    nx2[:, 2 * k:2 * k + 2],
                    lhsT=xsq[:, k * 128:(k + 1) * 128],
                    rhs=ones_f[:, 0:2],
                    start=True, stop=False,
                )
                nc.tensor.matmul(
                    nx2[:, 2 * k:2 * k + 2],
                    lhsT=x_sb[:, k * 128:(k + 1) * 128],
                    rhs=mu2bc[:, 0:2],
                    start=False, stop=True,
                )
            tn = small.tile([128, CH], F32, tag="tn")
            nc.scalar.activation(
                tn[:], nx2[:].rearrange("p (k two) -> p k two", two=2)[:, :, 0], AF.Ln
            )
            rx5 = persist.tile([128, CH], F32)
            nc.scalar.activation(rx5[:], tn[:], AF.Exp, bias=ln02[:], scale=-0.5)
            nrx5 = persist.tile([128, CH], F32)
            nc.vector.tensor_scalar_mul(nrx5[:], rx5[:], -H_BAND)

            # ---------------- main loop ----------------
            ssums = persist.tile([128, CH], F32)
            for k in range(CH):
                lhs = xcen[:, k * 128:(k + 1) * 128]
                pm = small.tile([128, 2], F32, tag="pm")
                rbt = rbpool.tile([128, N], F32, tag="rb")
                for h in range(2):
                    ps = mmpool.tile([128, HALF], F32, tag="mm")
                    for j in range(4):
                        c0 = h * HALF + j * 512
                        nc.tensor.matmul(
                            ps[:, j * 512:(j + 1) * 512],
                            lhsT=lhs,
                            rhs=yhat[:, c0:c0 + 512],
                            start=True, stop=True,
                        )
                    init = -3.0e38 if h == 0 else pm[:, 0:1]
                    # rb = copy(ps); pm[:,h] = max(row-max(ps), init)
                    nc.vector._custom_dve(
                        TENSOR_MASK_REDUCE,
                        out=rbt[:, h * HALF:(h + 1) * HALF],
                        in0=ps[:],
                        in1=c3big[:],
                        s0=0.0,
                        s1=init,
                        imm2=1.0,
                        accum_out=pm[:, h:h + 1],
                    )

                # per-row constants: aa = rx5 / (1.001 - 5*rx5*pmax),
                # bb = -aa*pmax   (gpsimd + fast reciprocal on DVE)
                den = small.tile([128, 1], F32, tag="den")
                nc.gpsimd.tensor_scalar(
                    den[:], pm[:, 1:2], nrx5[:, k:k + 1], 1.0 + EPS_MIN,
                    OP.mult, OP.add,
                )
                rec = small.tile([128, 1], F32, tag="rec")
                nc.vector.reciprocal_approx_fast(rec[:], den[:])
                aa = small.tile([128, 1], F32, tag="aa")
                nc.gpsimd.tensor_scalar(aa[:], rec[:], rx5[:, k:k + 1], None, OP.mult)
                bb = small.tile([128, 1], F32, tag="bb")
                nc.gpsimd.tensor_scalar(
                    bb[:], aa[:], pm[:, 1:2], -1.0, OP.mult, OP.mult
                )

                es = espool.tile([128, N], BF16, tag="es")
                nc.scalar.activation(
                    es[:], rbt[:], AF.Exp, bias=bb[:], scale=aa[:],
                    accum_out=ssums[:, k:k + 1],
                )

            # ---------------- finalize ----------------
            nc.sync.dma_start(out=out_d[:], in_=ssums[:])

    nc.compile()
    return nc


def _get_nc():
    if "nc" not in _NC_CACHE:
        _NC_CACHE["nc"] = build_nc()
    return _NC_CACHE["nc"]


def make_in_maps(X_features, Y_features):
    X = np.ascontiguousarray(np.asarray(X_features, np.float32).reshape(B, C, N))
    Y = np.ascontiguousarray(np.asarray(Y_features, np.float32).reshape(B, C, N))
    in_maps = []
    for c in range(8):
        b, h = divmod(c, 2)
        in_maps.append({
            "Xh": np.ascontiguousarray(X[b, :, h * NX:(h + 1) * NX]),
            "Yb": Y[b],
        })
    return in_maps


def combine(results):
    """results: list of 8 dicts with 'out' [128, CH] = S' per row."""
    out = np.empty(B, np.float32)
    for b in range(B):
        tot = 0.0
        for h in range(2):
            s = results[2 * b + h]["out"].astype(np.float64)
            tot += (1.0 / s).sum()
        out[b] = -np.log(tot / N)
    return out


def kernel(X_features, Y_features):
    nc = _get_nc()
    in_maps = make_in_maps(X_features, Y_features)
    res = run_bass_kernel_spmd(nc, in_maps, core_ids=list(range(8)))
    return combine(res.results)


if __name__ == "__main__":
    rng = np.random.default_rng(0)
    X = rng.standard_normal((B, C, 64, 64)).astype(np.float32)
    Y = rng.standard_normal((B, C, 64, 64)).astype(np.float32)
    print(kernel(X_features=X, Y_features=Y))


# revision 6
# speedup vs baseline: 1.0772x; 1.0772x over previous
"""Trainium2 Bass kernel for a contextual loss (cosine-distance softmin loss).

Math (per batch b):
  mu_c      = mean_n Y[b,c,n]
  xc = X-mu, yc = Y-mu                      (centered, [C,N])
  t[i,j]    = <xc_i, yc_j/||yc_j||>         (bf16 matmul, K=C=64)
  s[i,j]    = rx_i * t[i,j]                 (rx = 1/||xc_i||)
  pm_i      = max_j t[i,j]
  a_i       = rx5_i / (1.001 - 5*rx5_i*pm_i)     (rx5 = 0.2*rx)
  S'_i      = sum_j exp(a_i*(t_ij - pm_i))
  loss_b    = -log(mean_i 1/S'_i)

Sharding: 8 cores = 4 batches x 2 row-halves. Each core gets its full-batch
Y [64,4096] and its half of X's columns [64,2048], returns S' as [128,16]
(partition p, chunk k  <->  row k*128+p). Host reduces to the [4] loss.

Layout trick: X and Y are DMA'd TWICE, into partitions 0-63 and 64-127.
All elementwise setup ops then run on 128 partitions at the same cost as
64, and the duplicated halves feed the PE row-tiling: two 128-row chunks
run CONCURRENTLY as K=64 sub-matmuls at tile_position (0,0) / (64,0),
sharing one weight-load slot each, which keeps the PE dense.

Column norms without pre-centering: ||y_j - mu||^2 = colsum(y^2) - 2 mu^T y_j
(+ ||mu||^2, ~2e-4 relative, dropped), via PSUM accumulation of two bf16
matmuls: (0.5*ones)^T @ y^2  then  (-mu)^T @ y  over the duplicated rows.

On-device pipeline per 128-row chunk:
  PE   : 4 bf16 matmul slots (2 chunks in parallel, K=64, N=512)
  DVE  : TENSOR_MASK_REDUCE fuses PSUM->SBUF copy with a running row-max
  GPSIMD + DVE: tiny per-row chain  den -> 1/den -> aa -> bb
  ACT  : one exp(aa*t + bb) over [128,4096] with accumulated row-sum
"""

import math

import numpy as np

import concourse.bacc as bacc
import concourse.mybir as mybir
from concourse.dve_ops import TENSOR_MASK_REDUCE
from concourse.bass_utils import run_bass_kernel_spmd
from concourse.mybir import ActivationFunctionType as AF, AluOpType as OP, AxisListType
from concourse.tile import TileContext

F32 = mybir.dt.float32
BF16 = mybir.dt.bfloat16

B, C, N = 4, 64, 4096          # batch, channels, spatial (64*64)
NX = N // 2                    # rows per core (half batch)
CH = NX // 128                 # 16 chunks of 128 rows
HALF = N // 2                  # column half processed per DVE op
H_BAND = 5.0
EPS_MIN = 1e-3
LN02 = math.log(0.2)           # fold the 1/H into rx via exp(... + ln(1/H))

_NC_CACHE = {}


def build_nc():
    nc = bacc.Bacc("TRN2", target_bir_lowering=False, debug=False, num_devices=8)
    x_d = nc.dram_tensor("Xh", [C, NX], F32, kind="ExternalInput")
    y_d = nc.dram_tensor("Yb", [C, N], F32, kind="ExternalInput")
    out_d = nc.dram_tensor("out", [128, CH], F32, kind="ExternalOutput")

    with TileContext(nc) as tc:
        with (
            tc.tile_pool(name="persist", bufs=1) as persist,
            tc.tile_pool(name="mm", bufs=1, space="PSUM") as mmpool,
            tc.tile_pool(name="rb", bufs=2) as rbpool,
            tc.tile_pool(name="es", bufs=2) as espool,
            tc.tile_pool(name="small", bufs=4) as small,
        ):
            # -------- load inputs, duplicated into both partition halves -----
            y2 = persist.tile([128, N], F32)
            NSL = 4
            SL = N // NSL
            for sl in range(NSL):
                c0 = sl * SL
                nc.sync.dma_start(out=y2[:C, c0:c0 + SL], in_=y_d[:, c0:c0 + SL])
                nc.sync.dma_start(out=y2[C:, c0:c0 + SL], in_=y_d[:, c0:c0 + SL])
            x2 = persist.tile([128, NX], F32)
            nc.sync.dma_start(out=x2[:C, :], in_=x_d[:])
            nc.sync.dma_start(out=x2[C:, :], in_=x_d[:])

            hones = persist.tile([128, 128], BF16)
            nc.vector.memset(hones[:], 0.5)
            c3big = persist.tile([128, 1], F32)
            nc.gpsimd.memset(c3big[:], 1.0e9)
            ln02 = persist.tile([128, 1], F32)
            nc.gpsimd.memset(ln02[:], LN02)

            # ---------------- y mean (overlapped with DMA slices) -------------
            mus = small.tile([128, NSL], F32, tag="mus")
            for sl in range(NSL):
                c0 = sl * SL
                nc.vector.reduce_sum(out=mus[:, sl:sl + 1],
                                     in_=y2[:, c0:c0 + SL], axis=AxisListType.X)
            musum = small.tile([128, 1], F32, tag="musum")
            nc.vector.reduce_sum(out=musum[:], in_=mus[:], axis=AxisListType.X)
            mu = small.tile([128, 1], F32, tag="mu")
            nc.vector.tensor_scalar_mul(mu[:], musum[:], 1.0 / N)

            # -mu broadcast along free dim (bf16, for the colsum corrections)
            nmubc = persist.tile([128, 128], BF16)
            nc.vector.tensor_scalar(nmubc[:], hones[:], mu[:], -2.0,
                                    OP.mult, OP.mult)

            # squares of raw y/x and a bf16 copy of y (no mu dependency)
            ysq = persist.tile([128, N], BF16)
            for h in range(2):
                nc.scalar.activation(
                    ysq[:, h * HALF:(h + 1) * HALF],
                    y2[:, h * HALF:(h + 1) * HALF], AF.Square,
                )
            ybf = persist.tile([128, N], BF16)
            nc.vector.tensor_copy(ybf[:], y2[:])
            xsq = persist.tile([128, NX], BF16)
            nc.scalar.activation(xsq[:], x2[:], AF.Square)

            # ---------------- ry broadcast -> yhat (bf16) ----------------
            # ny2_j = 0.5*colsum128(y^2) - mu^T y_j via PSUM accumulation,
            # replicated down 128 partitions; ry = exp(-0.5*ln(ny2)).
            yhat = persist.tile([128, N], BF16)
            ry_bc = persist.tile([128, N], F32)
            for h in range(2):
                ps = mmpool.tile([128, HALF], F32, tag="mmA" if h == 0 else "mmB")
                for j in range(4):
                    c0 = h * HALF + j * 512
                    nc.tensor.matmul(
                        ps[:, j * 512:(j + 1) * 512],
                        lhsT=hones[:],
                        rhs=ysq[:, c0:c0 + 512],
                        start=True, stop=False,
                    )
                    nc.tensor.matmul(
                        ps[:, j * 512:(j + 1) * 512],
                        lhsT=nmubc[:],
                        rhs=ybf[:, c0:c0 + 512],
                        start=False, stop=True,
                    )
                tln = espool.tile([128, HALF], F32, tag="es")
                nc.scalar.activation(tln[:], ps[:], AF.Ln)
                nc.scalar.activation(
                    ry_bc[:, h * HALF:(h + 1) * HALF], tln[:], AF.Exp, scale=-0.5
                )
                # yhat half: (y - mu) * ry, cast to bf16, in quarters so the
                # first main-loop matmuls can start early
                QW = HALF // 2
                for q in range(2):
                    c0 = h * HALF + q * QW
                    nc.vector.scalar_tensor_tensor(
                        yhat[:, c0:c0 + QW],
                        in0=y2[:, c0:c0 + QW],
                        scalar=mu[:],
                        in1=ry_bc[:, c0:c0 + QW],
                        op0=OP.subtract,
                        op1=OP.mult,
                    )

            # ---------------- x side: xcen (bf16) + rx5 ----------------
            xcen = persist.tile([128, NX], BF16)
            nc.vector.tensor_scalar(xcen[:], x2[:], mu[:], None, OP.subtract)

            # nx2 = ||x_i - mu||^2 in [128 rows, chunk] layout, PSUM-accumulated
            # over the duplicated 128 rows: (x^2)^T @ 0.5 + x^T @ (-mu)
            honcol = persist.tile([128, 2], BF16)
            nc.vector.memset(honcol[:], 0.5)
            nmucol = persist.tile([128, 2], BF16)
            nc.vector.tensor_scalar(nmucol[:], honcol[:], mu[:], -2.0,
                                    OP.mult, OP.mult)
            nx2 = mmpool.tile([128, 2 * CH], F32, tag="mmA")
            for k in range(CH):
                nc.tensor.matmul(
                    nx2[:, 2 * k:2 * k + 2],
                    lhsT=xsq[:, k * 128:(k + 1) * 128],
                    rhs=honcol[:],
                    start=True, stop=False,
                )
                nc.tensor.matmul(
                    nx2[:, 2 * k:2 * k + 2],
                    lhsT=xcen[:, k * 128:(k + 1) * 128],
                    rhs=nmucol[:],
                    start=False, stop=True,
                )
            tn = small.tile([128, CH], F32, tag="tn")
            nc.scalar.activation(
                tn[:], nx2[:].rearrange("p (k two) -> p k two", two=2)[:, :, 0], AF.Ln
            )
            rx5 = persist.tile([128, CH], F32)
            nc.scalar.activation(rx5[:], tn[:], AF.Exp, bias=ln02[:], scale=-0.5)
            nrx5 = persist.tile([128, CH], F32)
            nc.vector.tensor_scalar_mul(nrx5[:], rx5[:], -H_BAND)

            # ---------------- main loop: chunk pairs via PE row tiling --------
            ssums = persist.tile([128, CH], F32)
            for kp in range(CH // 2):
                kA, kB = 2 * kp, 2 * kp + 1
                lhsA = xcen[0:C, kA * 128:(kA + 1) * 128]
                lhsB = xcen[C:128, kB * 128:(kB + 1) * 128]
                pmA = small.tile([128, 2], F32, tag="pmA", name="pmA")
                pmB = small.tile([128, 2], F32, tag="pmB", name="pmB")
                rbA = rbpool.tile([128, N], F32, tag="rbA", name="rbA")
                rbB = rbpool.tile([128, N], F32, tag="rbB", name="rbB")
                pms, rbts = [pmA, pmB], [rbA, rbB]
                for h in range(2):
                    psA = mmpool.tile([128, HALF], F32, tag="mmA")
                    psB = mmpool.tile([128, HALF], F32, tag="mmB")
                    for j in range(4):
                        c0 = h * HALF + j * 512
                        nc.tensor.matmul(
                            psA[:, j * 512:(j + 1) * 512],
                            lhsT=lhsA,
                            rhs=yhat[0:C, c0:c0 + 512],
                            start=True, stop=True,
                            tile_position=(0, 0),
                        )
                        nc.tensor.matmul(
                            psB[:, j * 512:(j + 1) * 512],
                            lhsT=lhsB,
                            rhs=yhat[C:128, c0:c0 + 512],
                            start=True, stop=True,
                            tile_position=(64, 0),
                        )
                    for i, ps in enumerate((psA, psB)):
                        init = -3.0e38 if h == 0 else pms[i][:, 0:1]
                        # rb = copy(ps); pm[:,h] = max(row-max(ps), init)
                        nc.vector._custom_dve(
                            TENSOR_MASK_REDUCE,
                            out=rbts[i][:, h * HALF:(h + 1) * HALF],
                            in0=ps[:],
                            in1=c3big[:],
                            s0=0.0,
                            s1=init,
                            imm2=1.0,
                            accum_out=pms[i][:, h:h + 1],
                        )

                for i, k in enumerate((kA, kB)):
                    # per-row constants: aa = rx5 / (1.001 - 5*rx5*pmax),
                    # bb = -aa*pmax   (gpsimd + fast reciprocal on DVE)
                    pm = pms[i]
                    den = small.tile([128, 1], F32, tag=f"den{i}", name="den")
                    nc.gpsimd.tensor_scalar(
                        den[:], pm[:, 1:2], nrx5[:, k:k + 1], 1.0 + EPS_MIN,
                        OP.mult, OP.add,
                    )
                    rec = small.tile([128, 1], F32, tag=f"rec{i}", name="rec")
                    nc.vector.reciprocal_approx_fast(rec[:], den[:])
                    aa = small.tile([128, 1], F32, tag=f"aa{i}", name="aa")
                    nc.gpsimd.tensor_scalar(aa[:], rec[:], rx5[:, k:k + 1],
                                            None, OP.mult)
                    bb = small.tile([128, 1], F32, tag=f"bb{i}", name="bb")
                    nc.gpsimd.tensor_scalar(
                        bb[:], aa[:], pm[:, 1:2], -1.0, OP.mult, OP.mult
                    )

                    es = espool.tile([128, N], BF16, tag="es")
                    nc.scalar.activation(
                        es[:], rbts[i][:], AF.Exp, bias=bb[:], scale=aa[:],
                        accum_out=ssums[:, k:k + 1],
                    )

            # ---------------- finalize ----------------
            nc.sync.dma_start(out=out_d[:], in_=ssums[:])

    nc.compile()
    return nc


def _get_nc():
    if "nc" not in _NC_CACHE:
        _NC_CACHE["nc"] = build_nc()
    return _NC_CACHE["nc"]


def make_in_maps(X_features, Y_features):
    X = np.ascontiguousarray(np.asarray(X_features, np.float32).reshape(B, C, N))
    Y = np.ascontiguousarray(np.asarray(Y_features, np.float32).reshape(B, C, N))
    in_maps = []
    for c in range(8):
        b, h = divmod(c, 2)
        in_maps.append({
            "Xh": np.ascontiguousarray(X[b, :, h * NX:(h + 1) * NX]),
            "Yb": Y[b],
        })
    return in_maps


def combine(results):
    """results: list of 8 dicts with 'out' [128, CH] = S' per row."""
    out = np.empty(B, np.float32)
    for b in range(B):
        tot = 0.0
        for h in range(2):
            s = results[2 * b + h]["out"].astype(np.float64)
            tot += (1.0 / s).sum()
        out[b] = -np.log(tot / N)
    return out


def kernel(X_features, Y_features):
    nc = _get_nc()
    in_maps = make_in_maps(X_features, Y_features)
    res = run_bass_kernel_spmd(nc, in_maps, core_ids=list(range(8)))
    return combine(res.results)


if __name__ == "__main__":
    rng = np.random.default_rng(0)
    X = rng.standard_normal((B, C, 64, 64)).astype(np.float32)
    Y = rng.standard_normal((B, C, 64, 64)).astype(np.float32)
    print(kernel(X_features=X, Y_features=Y))


# revision 8
# speedup vs baseline: 1.1296x; 1.0486x over previous
"""Trainium2 Bass kernel for a contextual loss (cosine-distance softmin loss).

Math (per batch b):
  mu_c      = mean_n Y[b,c,n]
  xc = X-mu, yc = Y-mu                      (centered, [C,N])
  t[i,j]    = <xc_i, yc_j/||yc_j||>         (bf16 matmul, K=C=64)
  s[i,j]    = rx_i * t[i,j]                 (rx = 1/||xc_i||)
  pm_i      = max_j t[i,j]
  a_i       = rx5_i / (1.001 - 5*rx5_i*pm_i)     (rx5 = 0.2*rx)
  S'_i      = sum_j exp(a_i*(t_ij - pm_i))
  loss_b    = -log(mean_i 1/S'_i)

Sharding: 8 cores = 4 batches x 2 row-halves. Each core gets its full-batch
Y [64,4096] and its half of X's columns [64,2048], returns S' as [128,16]
(partition p, chunk k  <->  row k*128+p). Host reduces to the [4] loss.

Layout trick: X and Y are DMA'd TWICE, into partitions 0-63 and 64-127.
All elementwise setup ops then run on 128 partitions at the same cost as
64, and the duplicated halves feed the PE row-tiling: two 128-row chunks
run CONCURRENTLY as K=64 sub-matmuls at tile_position (0,0) / (64,0),
sharing one weight-load slot each, which keeps the PE dense.

Column norms without pre-centering: ||y_j - mu||^2 = colsum(y^2) - 2 mu^T y_j
(+ ||mu||^2, ~2e-4 relative, dropped), via PSUM accumulation of two bf16
matmuls: (0.5*ones)^T @ y^2  then  (-mu)^T @ y  over the duplicated rows.

On-device pipeline per 128-row chunk:
  PE   : 4 bf16 matmul slots (2 chunks in parallel, K=64, N=512)
  DVE  : TENSOR_MASK_REDUCE fuses PSUM->SBUF copy with a running row-max
  GPSIMD + DVE: tiny per-row chain  den -> 1/den -> aa -> bb
  ACT  : one exp(aa*t + bb) over [128,4096] with accumulated row-sum
"""

import math

import numpy as np

import concourse.bacc as bacc
import concourse.mybir as mybir
from concourse.dve_ops import TENSOR_MASK_REDUCE
from concourse.bass_utils import run_bass_kernel_spmd
from concourse.mybir import ActivationFunctionType as AF, AluOpType as OP, AxisListType
from concourse.tile import TileContext

F32 = mybir.dt.float32
BF16 = mybir.dt.bfloat16

B, C, N = 4, 64, 4096          # batch, channels, spatial (64*64)
NX = N // 2                    # rows per core (half batch)
CH = NX // 128                 # 16 chunks of 128 rows
HALF = N // 2                  # column half processed per DVE op
H_BAND = 5.0
EPS_MIN = 1e-3
LN02 = math.log(0.2)           # fold the 1/H into rx via exp(... + ln(1/H))

_NC_CACHE = {}


def build_nc():
    nc = bacc.Bacc("TRN2", target_bir_lowering=False, debug=False, num_devices=8)
    x_d = nc.dram_tensor("Xh", [C, NX], F32, kind="ExternalInput")
    y_d = nc.dram_tensor("Yb", [C, N], F32, kind="ExternalInput")
    out_d = nc.dram_tensor("out", [128, 2 * CH], F32, kind="ExternalOutput")

    with TileContext(nc) as tc:
        with (
            tc.tile_pool(name="persist", bufs=1) as persist,
            tc.tile_pool(name="mm", bufs=1, space="PSUM") as mmpool,
            tc.tile_pool(name="rb", bufs=2) as rbpool,
            tc.tile_pool(name="es", bufs=2) as espool,
            tc.tile_pool(name="small", bufs=4) as small,
        ):
            # -------- load inputs, duplicated into both partition halves -----
            y2 = persist.tile([128, N], F32)
            NSL = 4
            SL = N // NSL
            for sl in range(NSL):
                c0 = sl * SL
                nc.sync.dma_start(out=y2[:C, c0:c0 + SL], in_=y_d[:, c0:c0 + SL])
                nc.sync.dma_start(out=y2[C:, c0:c0 + SL], in_=y2[:C, c0:c0 + SL])
            x2 = persist.tile([128, NX], F32)
            nc.sync.dma_start(out=x2[:C, :], in_=x_d[:])
            nc.sync.dma_start(out=x2[C:, :], in_=x2[:C, :])

            hones = persist.tile([128, 128], BF16)
            nc.vector.memset(hones[:], 0.5)
            c3big = persist.tile([128, 1], F32)
            nc.gpsimd.memset(c3big[:], 1.0e9)
            ln02 = persist.tile([128, 1], F32)
            nc.gpsimd.memset(ln02[:], LN02)

            # ---------------- y mean (overlapped with DMA slices) -------------
            mus = small.tile([128, NSL], F32, tag="mus")
            for sl in range(NSL):
                c0 = sl * SL
                nc.vector.reduce_sum(out=mus[:, sl:sl + 1],
                                     in_=y2[:, c0:c0 + SL], axis=AxisListType.X)
            musum = small.tile([128, 1], F32, tag="musum")
            nc.vector.reduce_sum(out=musum[:], in_=mus[:], axis=AxisListType.X)
            mu = small.tile([128, 1], F32, tag="mu")
            nc.vector.tensor_scalar_mul(mu[:], musum[:], 1.0 / N)

            # -mu broadcast along free dim (bf16, for the colsum corrections)
            nmubc = persist.tile([128, 128], BF16)
            nc.vector.tensor_scalar(nmubc[:], hones[:], mu[:], -2.0,
                                    OP.mult, OP.mult)

            # squares of raw y/x and a bf16 copy of y (no mu dependency)
            ysq = persist.tile([128, N], BF16)
            for h in range(2):
                nc.scalar.activation(
                    ysq[:, h * HALF:(h + 1) * HALF],
                    y2[:, h * HALF:(h + 1) * HALF], AF.Square,
                )
            xsq = persist.tile([128, NX], BF16)
            nc.scalar.activation(xsq[:], x2[:], AF.Square)
            ybf = persist.tile([128, N], BF16)
            nc.vector.tensor_copy(ybf[:], y2[:])

            # ---------------- ry broadcast -> yhat (bf16) ----------------
            # ny2_j = 0.5*colsum128(y^2) - mu^T y_j via PSUM accumulation,
            # replicated down 128 partitions; ry = exp(-0.5*ln(ny2)).
            yhat = persist.tile([128, N], BF16)
            ry_bc = persist.tile([128, N], F32)
            for h in range(2):
                ps = mmpool.tile([128, HALF], F32, tag="mmA" if h == 0 else "mmB")
                for j in range(4):
                    c0 = h * HALF + j * 512
                    nc.tensor.matmul(
                        ps[:, j * 512:(j + 1) * 512],
                        lhsT=hones[:],
                        rhs=ysq[:, c0:c0 + 512],
                        start=True, stop=False,
                    )
                    nc.tensor.matmul(
                        ps[:, j * 512:(j + 1) * 512],
                        lhsT=nmubc[:],
                        rhs=ybf[:, c0:c0 + 512],
                        start=False, stop=True,
                    )
                tln = espool.tile([128, HALF], F32, tag="es")
                nc.scalar.activation(tln[:], ps[:], AF.Ln)
                nc.scalar.activation(
                    ry_bc[:, h * HALF:(h + 1) * HALF], tln[:], AF.Exp, scale=-0.5
                )
                # yhat half: (y - mu) * ry, cast to bf16, in quarters so the
                # first main-loop matmuls can start early
                QW = HALF // 2
                for q in range(2):
                    c0 = h * HALF + q * QW
                    nc.vector.scalar_tensor_tensor(
                        yhat[:, c0:c0 + QW],
                        in0=y2[:, c0:c0 + QW],
                        scalar=mu[:],
                        in1=ry_bc[:, c0:c0 + QW],
                        op0=OP.subtract,
                        op1=OP.mult,
                    )

            # ---------------- x side: xcen (bf16) + rx5 ----------------
            xcen = persist.tile([128, NX], BF16)
            nc.vector.tensor_scalar(xcen[:], x2[:], mu[:], None, OP.subtract)

            # nx2 = ||x_i - mu||^2 in [128 rows, chunk] layout, PSUM-accumulated
            # over the duplicated 128 rows: (x^2)^T @ 0.5 + x^T @ (-mu)
            honcol = persist.tile([128, 2], BF16)
            nc.vector.memset(honcol[:], 0.5)
            nmucol = persist.tile([128, 2], BF16)
            nc.vector.tensor_scalar(nmucol[:], honcol[:], mu[:], -2.0,
                                    OP.mult, OP.mult)
            nx2 = mmpool.tile([128, 2 * CH], F32, tag="mmA")
            for k in range(CH):
                nc.tensor.matmul(
                    nx2[:, 2 * k:2 * k + 2],
                    lhsT=xsq[:, k * 128:(k + 1) * 128],
                    rhs=honcol[:],
                    start=True, stop=False,
                )
                nc.tensor.matmul(
                    nx2[:, 2 * k:2 * k + 2],
                    lhsT=xcen[:, k * 128:(k + 1) * 128],
                    rhs=nmucol[:],
                    start=False, stop=True,
                )
            tn = small.tile([128, CH], F32, tag="tn")
            nc.scalar.activation(
                tn[:], nx2[:].rearrange("p (k two) -> p k two", two=2)[:, :, 0], AF.Ln
            )
            rx5 = persist.tile([128, CH], F32)
            nc.scalar.activation(rx5[:], tn[:], AF.Exp, bias=ln02[:], scale=-0.5)
            rxr = small.tile([128, CH], F32, tag="rxr", name="rxr")
            nc.vector.reciprocal_approx_fast(rxr[:], rx5[:])
            cpm = persist.tile([128, CH], F32)
            nc.vector.tensor_scalar_mul(cpm[:], rxr[:], -0.2 * (1.0 + EPS_MIN))

            # ---------------- main loop: chunk pairs via PE row tiling --------
            ssums = persist.tile([128, 2 * CH], F32)
            for kp in range(CH // 2):
                kA, kB = 2 * kp, 2 * kp + 1
                lhsA = xcen[0:C, kA * 128:(kA + 1) * 128]
                lhsB = xcen[C:128, kB * 128:(kB + 1) * 128]
                pmP = small.tile([128, 4], F32, tag="pmP", name="pmP")
                rbA = rbpool.tile([128, N], F32, tag="rbA", name="rbA")
                rbB = rbpool.tile([128, N], F32, tag="rbB", name="rbB")
                rbts = [rbA, rbB]
                for h in range(2):
                    psA = mmpool.tile([128, HALF], F32, tag="mmA", name="psA")
                    psB = mmpool.tile([128, HALF], F32, tag="mmB", name="psB")
                    for j in range(4):
                        c0 = h * HALF + j * 512
                        nc.tensor.matmul(
                            psA[:, j * 512:(j + 1) * 512],
                            lhsT=lhsA,
                            rhs=yhat[0:C, c0:c0 + 512],
                            start=True, stop=True,
                            tile_position=(0, 0),
                        )
                        nc.tensor.matmul(
                            psB[:, j * 512:(j + 1) * 512],
                            lhsT=lhsB,
                            rhs=yhat[C:128, c0:c0 + 512],
                            start=True, stop=True,
                            tile_position=(64, 0),
                        )
                    for i, ps in enumerate((psA, psB)):
                        init = -3.0e38 if h == 0 else pmP[:, 2 * i:2 * i + 1]
                        # rb = copy(ps); pm = max(row-max(ps), init)
                        nc.vector._custom_dve(
                            TENSOR_MASK_REDUCE,
                            out=rbts[i][:, h * HALF:(h + 1) * HALF],
                            in0=ps[:],
                            in1=c3big[:],
                            s0=0.0,
                            s1=init,
                            imm2=1.0,
                            accum_out=pmP[:, 2 * i + h:2 * i + h + 1],
                        )

                # pair-batched per-row constants ([128,2] ops):
                #   aa = -0.2/(pm + c),  c = -0.2002/rx5,  bb = -aa*pm
                pm2 = pmP[:, 1::2]
                tden = small.tile([128, 2], F32, tag="tden", name="tden")
                nc.gpsimd.tensor_tensor(tden[:], pm2, cpm[:, kA:kB + 1], OP.add)
                rec2 = small.tile([128, 2], F32, tag="rec2", name="rec2")
                nc.vector.reciprocal_approx_fast(rec2[:], tden[:])
                aa2 = small.tile([128, 2], F32, tag="aa2", name="aa2")
                nc.gpsimd.tensor_scalar_mul(aa2[:], rec2[:], -0.2)
                bb2 = small.tile([128, 2], F32, tag="bb2", name="bb2")
                nc.vector.scalar_tensor_tensor(
                    bb2[:], in0=aa2[:], scalar=-1.0, in1=pm2,
                    op0=OP.mult, op1=OP.mult,
                )

                for i, k in enumerate((kA, kB)):
                    for h in range(2):
                        es = espool.tile([128, HALF], BF16, tag="es", name="es")
                        nc.scalar.activation(
                            es[:], rbts[i][:, h * HALF:(h + 1) * HALF], AF.Exp,
                            bias=bb2[:, i:i + 1], scale=aa2[:, i:i + 1],
                            accum_out=ssums[:, 2 * k + h:2 * k + h + 1],
                        )

            # ---------------- finalize ----------------
            nc.sync.dma_start(out=out_d[:], in_=ssums[:])

    nc.compile()
    return nc


def _get_nc():
    if "nc" not in _NC_CACHE:
        _NC_CACHE["nc"] = build_nc()
    return _NC_CACHE["nc"]


def make_in_maps(X_features, Y_features):
    X = np.ascontiguousarray(np.asarray(X_features, np.float32).reshape(B, C, N))
    Y = np.ascontiguousarray(np.asarray(Y_features, np.float32).reshape(B, C, N))
    in_maps = []
    for c in range(8):
        b, h = divmod(c, 2)
        in_maps.append({
            "Xh": np.ascontiguousarray(X[b, :, h * NX:(h + 1) * NX]),
            "Yb": Y[b],
        })
    return in_maps


def combine(results):
    """results: list of 8 dicts with 'out' [128, CH] = S' per row."""
    out = np.empty(B, np.float32)
    for b in range(B):
        tot = 0.0
        for h in range(2):
            s2 = results[2 * b + h]["out"].astype(np.float64)
            s = s2[:, 0::2] + s2[:, 1::2]
            tot += (1.0 / s).sum()
        out[b] = -np.log(tot / N)
    return out


def kernel(X_features, Y_features):
    nc = _get_nc()
    in_maps = make_in_maps(X_features, Y_features)
    res = run_bass_kernel_spmd(nc, in_maps, core_ids=list(range(8)))
    return combine(res.results)


if __name__ == "__main__":
    rng = np.random.default_rng(0)
    X = rng.standard_normal((B, C, 64, 64)).astype(np.float32)
    Y = rng.standard_normal((B, C, 64, 64)).astype(np.float32)
    print(kernel(X_features=X, Y_features=Y))
